# revision 6
# baseline (speedup 1.0000x reference)
"""Deformable (region-aware) matmul for Trainium2, data-parallel over batch.

out[b,o,h,w] = sum_r sum_c mat0[b,c,h,w] * mat1[o,c,r] * Alpha[r] * mask[r,h,w]

Shapes: B=8, C=256, H=W=64, O=256, R=8.  One batch per NeuronCore (8 cores).

Per-core algorithm: fold the (region, channel) pair into one contraction axis
K = R*C = 2048 (16 partition tiles of 128).  K-tile t <-> (r = t//2, half =
t%2).  The activation tile for t is X_t[k, p] = mat0[half*128+k, p] *
mask[r, p], produced on the Vector engine as a bf16 multiply (2x mode)
against a host-prebroadcast mask.  Weights W_t[k, o] = mat1[o, half*128+k, r]
* Alpha[r] are host-transposed to lhsT layout.  The Tensor engine accumulates
out[o, p] = sum_t W_t.T @ X_t in PSUM over the 16 K-tiles.

Schedule: per 1024-pixel chunk the matmuls run K-tile-OUTERMOST across the
four PSUM banks (m0n0, m0n1, m1n0, m1n1), so the PE consumes xt tiles in
exactly the order the Vector engine produces them (4 MMs of 864 ns per
594 ns mul) -- the PE never stalls on the DVE after the first tile.  The
last chunk instead runs bank-sequential (t inner) so the banks stop
staggered and the output tail is one copy+DMA deep.  A short zero-input
warmup keeps the PE HAM activity window busy from t=0 so the real stream
runs at 2.4 GHz.  Output is stored bf16 (cast in the PSUM->SBUF copy) to
halve output HBM traffic; the host converts back to fp32.
"""

import numpy as np
import ml_dtypes

B, C, H, W_ = 8, 256, 64, 64
O, R = 256, 8
P = H * W_            # 4096 pixels
KT = 2 * R            # 16 K-tiles of 128
PCHUNK = 1024         # pixel chunk per pipeline step
NCHUNK = P // PCHUNK  # 4
MMN = 512             # moving free dim per matmul (one PSUM bank of fp32)
NWARM = 3             # warmup matmuls (bridge trigger latency at body start)

_CACHE = {}


def _build():
    import concourse.bacc as bacc
    import concourse.tile as tile
    import concourse.mybir as mybir

    bf16 = mybir.dt.bfloat16
    f32 = mybir.dt.float32

    nc = bacc.Bacc(
        "TRN2",
        target_bir_lowering=False,
        debug=False,
        enable_asserts=False,
        num_devices=8,
    )
    # Per-core inputs (host-prepped layouts, see kernel()):
    #   x[k, half, p]  = mat0[b, half*128+k, p]            (bf16)
    #   w[k, t, o]     = mat1[o, c(t,k), r(t)] * Alpha     (bf16, lhsT layout)
    #   mb[r, q, p]    = mask[r, p] for all q              (bf16, row-broadcast)
    x_d = nc.dram_tensor("x", [128, 2, P], bf16, kind="ExternalInput")
    w_d = nc.dram_tensor("w", [128, KT, O], bf16, kind="ExternalInput")
    mb_d = nc.dram_tensor("mb", [R, 128, P], bf16, kind="ExternalInput")
    y_d = nc.dram_tensor("y", [2, 128, P], bf16, kind="ExternalOutput")

    with tile.TileContext(nc) as tc:
        with (
            tc.tile_pool(name="const", bufs=1) as cpool,
            tc.tile_pool(name="xcp", bufs=2) as xcpool,
            tc.tile_pool(name="mbp", bufs=2) as mbpool,
            tc.tile_pool(name="xp", bufs=2) as xpool,
            tc.tile_pool(name="psp", bufs=8, space="PSUM") as pspool,
            tc.tile_pool(name="yp", bufs=4) as ypool,
        ):
            # --- PE warmup: matmuls with no input deps (operands are zeroed
            # SBUF, output PSUM never read); they run while the prologue DMAs
            # are in flight and lift the HAM clock gate from 1.2 to 2.4 GHz.
            warm_w = cpool.tile([128, 128], bf16, tag="ww")
            warm_x = cpool.tile([128, MMN], bf16, tag="wx")
            nc.gpsimd.memset(warm_w[:], 0.0)
            nc.gpsimd.memset(warm_x[:], 0.0)
            warm_ps = pspool.tile([128, MMN], f32, tag="ps")
            for i in range(NWARM):
                nc.tensor.matmul(
                    warm_ps[:], warm_w[:], warm_x[:], start=True, stop=True
                )

            w_sb = cpool.tile([128, KT, O], bf16, tag="w")

            def dma_w(tq):  # one DMA of 2 K-tiles of weights (128 KiB)
                nc.scalar.dma_start(
                    out=w_sb[:, 2 * tq : 2 * (tq + 1), :],
                    in_=w_d[:, 2 * tq : 2 * (tq + 1), :],
                )

            for ci in range(NCHUNK):
                sl = slice(ci * PCHUNK, (ci + 1) * PCHUNK)
                # x chunk, split by half so mul t=0 can start a bit earlier
                x_sb = xcpool.tile([128, 2, PCHUNK], bf16, tag="xc")
                for half in range(2):
                    nc.scalar.dma_start(
                        out=x_sb[:, half, :], in_=x_d[:, half, sl]
                    )
                mb_sb = mbpool.tile([128, R, PCHUNK], bf16, tag="mb")
                if ci == 0:
                    # Interleave weight pieces behind the mask rows in need
                    # order: the PE consumes K-tile t at ~t*0.95us, needing
                    # mask row t//2 and weight piece t//2; issue them in
                    # lockstep so neither starves the stream.
                    for r in range(R):
                        nc.sync.dma_start(
                            out=mb_sb[:, r, :], in_=mb_d[r, :, sl]
                        )
                        dma_w(r)
                else:
                    # later chunks: coarser mask pieces (2 rows each)
                    for rq in range(R // 2):
                        nc.sync.dma_start(
                            out=mb_sb[:, 2 * rq : 2 * (rq + 1), :],
                            in_=mb_d[2 * rq : 2 * (rq + 1), :, sl].rearrange(
                                "r q p -> q r p"
                            ),
                        )
                xt = xpool.tile([128, KT, PCHUNK], bf16, tag="xt")
                for t in range(KT):
                    r, half = t // 2, t % 2
                    nc.vector.tensor_mul(
                        xt[:, t, :], x_sb[:, half, :], mb_sb[:, r, :]
                    )

                nn_banks = PCHUNK // MMN  # 2
                if ci < NCHUNK - 1:
                    # K-outer, bank-interleaved: PE follows the DVE tile
                    # production order with zero stalls.
                    banks = {}
                    for m in range(2):
                        for nn in range(nn_banks):
                            banks[(m, nn)] = pspool.tile(
                                [128, MMN], f32, tag="ps", name="ps"
                            )
                    for t in range(KT):
                        for m in range(2):
                            for nn in range(nn_banks):
                                nsl = slice(nn * MMN, (nn + 1) * MMN)
                                nc.tensor.matmul(
                                    banks[(m, nn)][:],
                                    w_sb[:, t, m * 128 : (m + 1) * 128],
                                    xt[:, t, nsl],
                                    start=(t == 0),
                                    stop=(t == KT - 1),
                                )
                    for m in range(2):
                        y_sb = ypool.tile([128, PCHUNK], bf16, tag="y")
                        for nn in range(nn_banks):
                            nc.scalar.copy(
                                y_sb[:, nn * MMN : (nn + 1) * MMN],
                                banks[(m, nn)][:],
                            )
                        nc.sync.dma_start(
                            out=y_d[m, :, sl], in_=y_sb[:]
                        )
                else:
                    # Last chunk: bank-sequential so the banks stop staggered
                    # and the tail is only one copy+DMA deep.
                    for m in range(2):
                        for nn in range(nn_banks):
                            nsl = slice(nn * MMN, (nn + 1) * MMN)
                            ps = pspool.tile([128, MMN], f32, tag="ps")
                            for t in range(KT):
                                nc.tensor.matmul(
                                    ps[:],
                                    w_sb[:, t, m * 128 : (m + 1) * 128],
                                    xt[:, t, nsl],
                                    start=(t == 0),
                                    stop=(t == KT - 1),
                                )
                            y_sb = ypool.tile([128, MMN], bf16, tag="y")
                            nc.scalar.copy(y_sb[:], ps[:])
                            st = ci * PCHUNK + nn * MMN
                            nc.sync.dma_start(
                                out=y_d[m, :, st : st + MMN], in_=y_sb[:]
                            )

    nc.compile()
    return nc


def _prep_inputs(mat0, mat1, mask, Alpha, use_alpha):
    bf = ml_dtypes.bfloat16
    m1 = mat1 * np.asarray(Alpha)[None, None, :] if int(use_alpha) else mat1
    # w[k, t, o] with t = r*2 + half, c = half*128 + k
    w = np.transpose(m1.reshape(O, 2, 128, R), (2, 3, 1, 0))  # [k, r, half, o]
    w_h = np.ascontiguousarray(w.reshape(128, KT, O)).astype(bf)
    # mb[r, q, p] = mask[r, p]
    mb_h = np.ascontiguousarray(
        np.broadcast_to(mask.reshape(R, 1, P), (R, 128, P))
    ).astype(bf)
    # x[b][k, half, p] = mat0[b, half*128+k, p]
    x_h = np.ascontiguousarray(
        np.transpose(mat0.reshape(B, 2, 128, P), (0, 2, 1, 3))
    ).astype(bf)
    return x_h, w_h, mb_h


def kernel(mat0, mat1, mask, Alpha, use_alpha, beta):
    from concourse import bass_utils

    mat0 = np.asarray(mat0, dtype=np.float32)
    mat1 = np.asarray(mat1, dtype=np.float32)
    mask = np.asarray(mask, dtype=np.float32)
    Alpha = np.asarray(Alpha, dtype=np.float32)

    if "nc" not in _CACHE:
        _CACHE["nc"] = _build()
    nc = _CACHE["nc"]

    x_h, w_h, mb_h = _prep_inputs(mat0, mat1, mask, Alpha, use_alpha)
    in_maps = [{"x": x_h[b], "w": w_h, "mb": mb_h} for b in range(B)]
    res = bass_utils.run_bass_kernel_spmd(nc, in_maps, core_ids=list(range(B)))
    _CACHE["last_res"] = res
    out = np.stack(
        [res.results[b]["y"].reshape(O, H, W_).astype(np.float32) for b in range(B)]
    )
    return out


# revision 7
# speedup vs baseline: 1.1652x; 1.1652x over previous
"""Deformable (region-aware) matmul for Trainium2, data-parallel over batch.

out[b,o,h,w] = sum_r sum_c mat0[b,c,h,w] * mat1[o,c,r] * Alpha[r] * mask[r,h,w]

Shapes: B=8, C=256, H=W=64, O=256, R=8.  One batch per NeuronCore (8 cores).

Per-core algorithm: fold the (region, channel) pair into one contraction axis
K = R*C = 2048 (16 partition tiles of 128).  K-tile t <-> (r = t//2, half =
t%2).  The activation tile for t is X_t[k, p] = mat0[half*128+k, p] *
mask[r, p], produced on the Vector engine as a bf16 multiply (2x mode)
against a host-prebroadcast mask.  Weights W_t[k, o] = mat1[o, half*128+k, r]
* Alpha[r] are host-transposed to lhsT layout.  The Tensor engine accumulates
out[o, p] = sum_t W_t.T @ X_t in PSUM over the 16 K-tiles.

Scheduling notes (from perfetto analysis):
- DMA triggers (DIRECT2D) cost ~6ns/descriptor serialized on the issuing
  sequencer, so bulk transfers use host layouts with 16KB-per-partition
  contiguous runs (128 descriptors per trigger instead of 1024).
- Tile dependencies are tile-granular, so chunk 0 splits x/mask/weights into
  separate tiles per piece for fine-grained readiness; later chunks use one
  big DMA each.
- The matmuls run K-tile-outermost across the four PSUM banks of a chunk so
  the PE consumes xt tiles in DVE production order (no producer stalls); the
  last chunk runs bank-sequential so the banks stop staggered and the output
  tail is one copy+DMA deep.
- Zero-input warmup matmuls keep the PE HAM activity window busy from the
  start of the body so the real stream runs at 2.4 GHz.
- Output is bf16 (cast in the PSUM->SBUF copy); host converts back to fp32.
"""

import numpy as np
import ml_dtypes

B, C, H, W_ = 8, 256, 64, 64
O, R = 256, 8
P = H * W_            # 4096 pixels
KT = 2 * R            # 16 K-tiles of 128
PCHUNK = 1024         # pixel chunk per pipeline step
NCHUNK = P // PCHUNK  # 4
MMN = 512             # moving free dim per matmul (one PSUM bank of fp32)
NWARM = 8             # warmup matmuls (~3.4us: bridge body start to first data)

_CACHE = {}


def _build():
    import concourse.bacc as bacc
    import concourse.tile as tile
    import concourse.mybir as mybir

    bf16 = mybir.dt.bfloat16
    f32 = mybir.dt.float32

    nc = bacc.Bacc(
        "TRN2",
        target_bir_lowering=False,
        debug=False,
        enable_asserts=False,
        num_devices=8,
    )
    # Per-core inputs (host-prepped layouts, see kernel()):
    #   x[k, ci, half, p]   = mat0[b, half*128+k, ci*1024+p]   (bf16)
    #   w[k, t, o]          = mat1[o, c(t,k), r(t)] * Alpha    (bf16, lhsT)
    #   mb0[r, q, p]        = mask[r, p] (chunk 0, row-granular pieces)
    #   mbr[q, ci, r, p]    = mask[r, ci*1024+p] (chunks 1.., 16KB descs)
    x_d = nc.dram_tensor("x", [128, NCHUNK, 2, PCHUNK], bf16, kind="ExternalInput")
    w_d = nc.dram_tensor("w", [128, KT, O], bf16, kind="ExternalInput")
    mb0_d = nc.dram_tensor("mb0", [R, 128, PCHUNK], bf16, kind="ExternalInput")
    mbr_d = nc.dram_tensor(
        "mbr", [128, NCHUNK - 1, R, PCHUNK], bf16, kind="ExternalInput"
    )
    y_d = nc.dram_tensor("y", [2, 128, P], bf16, kind="ExternalOutput")

    with tile.TileContext(nc) as tc:
        with (
            tc.tile_pool(name="const", bufs=1) as cpool,
            tc.tile_pool(name="m0p", bufs=1) as m0pool,
            tc.tile_pool(name="xcp", bufs=2) as xcpool,
            tc.tile_pool(name="mbp", bufs=2) as mbpool,
            tc.tile_pool(name="xp", bufs=2) as xpool,
            tc.tile_pool(name="psp", bufs=8, space="PSUM") as pspool,
            tc.tile_pool(name="yp", bufs=4) as ypool,
        ):
            # --- PE warmup: matmuls on zeroed SBUF (output PSUM never read);
            # they run while the prologue DMAs are in flight and lift the HAM
            # clock gate from 1.2 to 2.4 GHz before the real stream begins.
            warm_w = cpool.tile([128, 128], bf16, tag="ww")
            warm_x = cpool.tile([128, MMN], bf16, tag="wx")
            nc.gpsimd.memset(warm_w[:], 0.0)
            nc.gpsimd.memset(warm_x[:], 0.0)
            warm_ps = pspool.tile([128, MMN], f32, tag="ps")
            for i in range(NWARM):
                nc.tensor.matmul(
                    warm_ps[:], warm_w[:], warm_x[:], start=True, stop=True
                )

            # Weights: two tiles so the first K-tiles are ready early.
            w_lo = cpool.tile([128, 4, O], bf16, tag="wlo")
            w_hi = cpool.tile([128, KT - 4, O], bf16, tag="whi")

            def w_sb(t):
                return w_lo[:, t, :] if t < 4 else w_hi[:, t - 4, :]

            # Chunk 0: row-granular mask tiles + half-granular x tiles for
            # the fastest possible pipeline fill.
            x0h = []
            for half in range(2):
                xh = xcpool.tile(
                    [128, PCHUNK], bf16, tag=f"x0h{half}", name="x0h", bufs=1
                )
                nc.scalar.dma_start(out=xh[:], in_=x_d[:, 0, half, :])
                if half == 0:
                    nc.scalar.dma_start(out=w_lo[:], in_=w_d[:, 0:4, :])
                else:
                    nc.scalar.dma_start(out=w_hi[:], in_=w_d[:, 4:KT, :])
                x0h.append(xh)
            mb0 = []
            for r in range(R):
                mrow = m0pool.tile([128, PCHUNK], bf16, tag=f"m0r{r}", name="m0r")
                nc.sync.dma_start(out=mrow[:], in_=mb0_d[r, :, :])
                mb0.append(mrow)

            for ci in range(NCHUNK):
                sl = slice(ci * PCHUNK, (ci + 1) * PCHUNK)
                if ci > 0:
                    x_sb = xcpool.tile([128, 2, PCHUNK], bf16, tag="xc")
                    nc.scalar.dma_start(out=x_sb[:], in_=x_d[:, ci, :, :])
                    mb_sb = mbpool.tile([128, R, PCHUNK], bf16, tag="mb")
                    nc.sync.dma_start(out=mb_sb[:], in_=mbr_d[:, ci - 1, :, :])

                xt = xpool.tile([128, KT, PCHUNK], bf16, tag="xt")
                for t in range(KT):
                    r, half = t // 2, t % 2
                    if ci == 0:
                        nc.vector.tensor_mul(xt[:, t, :], x0h[half][:], mb0[r][:])
                    else:
                        nc.vector.tensor_mul(
                            xt[:, t, :], x_sb[:, half, :], mb_sb[:, r, :]
                        )

                nn_banks = PCHUNK // MMN  # 2
                if ci < NCHUNK - 1:
                    # K-outer, bank-interleaved: PE follows the DVE tile
                    # production order with zero stalls.
                    banks = {}
                    for m in range(2):
                        for nn in range(nn_banks):
                            banks[(m, nn)] = pspool.tile(
                                [128, MMN], f32, tag="ps", name="ps"
                            )
                    for t in range(KT):
                        for m in range(2):
                            for nn in range(nn_banks):
                                nsl = slice(nn * MMN, (nn + 1) * MMN)
                                nc.tensor.matmul(
                                    banks[(m, nn)][:],
                                    w_sb(t)[:, m * 128 : (m + 1) * 128],
                                    xt[:, t, nsl],
                                    start=(t == 0),
                                    stop=(t == KT - 1),
                                )
                    for m in range(2):
                        y_sb = ypool.tile([128, PCHUNK], bf16, tag="y")
                        for nn in range(nn_banks):
                            nc.scalar.copy(
                                y_sb[:, nn * MMN : (nn + 1) * MMN],
                                banks[(m, nn)][:],
                            )
                        nc.gpsimd.dma_start(out=y_d[m, :, sl], in_=y_sb[:])
                else:
                    # Last chunk: bank-sequential so the banks stop staggered
                    # and the tail is only one copy+DMA deep.
                    for m in range(2):
                        for nn in range(nn_banks):
                            nsl = slice(nn * MMN, (nn + 1) * MMN)
                            ps = pspool.tile([128, MMN], f32, tag="ps")
                            for t in range(KT):
                                nc.tensor.matmul(
                                    ps[:],
                                    w_sb(t)[:, m * 128 : (m + 1) * 128],
                                    xt[:, t, nsl],
                                    start=(t == 0),
                                    stop=(t == KT - 1),
                                )
                            y_sb = ypool.tile([128, MMN], bf16, tag="ylast", name="y_sb")
                            nc.scalar.copy(y_sb[:], ps[:])
                            st = ci * PCHUNK + nn * MMN
                            nc.gpsimd.dma_start(
                                out=y_d[m, :, st : st + MMN], in_=y_sb[:]
                            )

    nc.compile()
    return nc


def _prep_inputs(mat0, mat1, mask, Alpha, use_alpha):
    bf = ml_dtypes.bfloat16
    m1 = mat1 * np.asarray(Alpha)[None, None, :] if int(use_alpha) else mat1
    # w[k, t, o] with t = r*2 + half, c = half*128 + k
    w = np.transpose(m1.reshape(O, 2, 128, R), (2, 3, 1, 0))  # [k, r, half, o]
    w_h = np.ascontiguousarray(w.reshape(128, KT, O)).astype(bf)
    mbf = mask.reshape(R, P).astype(bf)
    # mb0[r, q, p] = mask[r, p] for chunk 0
    mb0_h = np.ascontiguousarray(
        np.broadcast_to(mbf[:, None, :PCHUNK], (R, 128, PCHUNK))
    )
    # mbr[q, ci, r, p] = mask[r, (ci+1)*1024 + p]
    mtail = mbf[:, PCHUNK:].reshape(R, NCHUNK - 1, PCHUNK)  # [r, ci, p]
    mbr_h = np.ascontiguousarray(
        np.broadcast_to(
            np.transpose(mtail, (1, 0, 2))[None], (128, NCHUNK - 1, R, PCHUNK)
        )
    )
    # x[b][k, ci, half, p] = mat0[b, half*128+k, ci*1024+p]
    x4 = mat0.reshape(B, 2, 128, NCHUNK, PCHUNK)
    x_h = np.ascontiguousarray(np.transpose(x4, (0, 2, 3, 1, 4))).astype(bf)
    return x_h, w_h, mb0_h, mbr_h


def kernel(mat0, mat1, mask, Alpha, use_alpha, beta):
    from concourse import bass_utils

    mat0 = np.asarray(mat0, dtype=np.float32)
    mat1 = np.asarray(mat1, dtype=np.float32)
    mask = np.asarray(mask, dtype=np.float32)
    Alpha = np.asarray(Alpha, dtype=np.float32)

    if "nc" not in _CACHE:
        _CACHE["nc"] = _build()
    nc = _CACHE["nc"]

    x_h, w_h, mb0_h, mbr_h = _prep_inputs(mat0, mat1, mask, Alpha, use_alpha)
    in_maps = [
        {"x": x_h[b], "w": w_h, "mb0": mb0_h, "mbr": mbr_h} for b in range(B)
    ]
    res = bass_utils.run_bass_kernel_spmd(nc, in_maps, core_ids=list(range(B)))
    _CACHE["last_res"] = res
    out = np.stack(
        [res.results[b]["y"].reshape(O, H, W_).astype(np.float32) for b in range(B)]
    )
    return out
